# revision 4
# baseline (speedup 1.0000x reference)
"""Trainium2 Bass kernel for nn_CtcHead (segment-reduce + 2-layer head + CE).

Sharding: 8 cores, core c -> (batch b = c//2, half h = c%2).
Key algebraic reduction: cell_states[b,s] = group_mean[b, indicator[b,s]]
with only 256 groups per batch, so the tanh-head/log-softmax/argmax run on
256 group rows per batch instead of 16384 tokens.  Per-token outputs are
gathers from a [256, 9] table via a one-hot matmul.

Matmuls run as fp32r (fp32 with 11-bit mantissa, round-to-nearest-even,
exact PSUM f32 accumulate).  Host pre-rounds float inputs to the fp32r
grid so tiles can be DMA'd directly into float32r SBUF tensors.
"""
import sys
import numpy as np

sys.path.insert(0, "/opt/trn_rl_repo")

B, S, H = 4, 4096, 768
G = 256          # indicator groups
T = 8            # NUM_TYPES
HALF = S // 2    # tokens per core
HT = HALF // 128  # token s-blocks per core (16)
HB = H // 128    # h-blocks (6)
N_CORES = 8

_CACHE = {}
TRACE = False


def _rnd_fp32r(x):
    """Round float32 array to the fp32r grid (11-bit mantissa, nearest-even)."""
    u = np.ascontiguousarray(x, np.float32).view(np.uint32).astype(np.uint64)
    lsb = (u >> np.uint64(12)) & np.uint64(1)
    u2 = (u + np.uint64(0x7FF) + lsb) & np.uint64(0xFFFFF000)
    return u2.astype(np.uint32).view(np.float32)


def _build_nc(nk):
    """Build the Bass program.  nk = number of 128-token k-tiles fed to the
    segment-sum (32: full batch per core)."""
    import concourse.bacc as bacc
    import concourse.tile as tile
    import concourse.mybir as mybir

    F32 = mybir.dt.float32
    F32R = mybir.dt.float32r
    AF = mybir.ActivationFunctionType
    OP = mybir.AluOpType
    AX = mybir.AxisListType

    nc = bacc.Bacc("TRN2", target_bir_lowering=False, debug=False,
                   num_devices=N_CORES)

    d_e = nc.dram_tensor("e", [nk * 128, H], F32, kind="ExternalInput")
    d_indcol = nc.dram_tensor("indcol", [128, nk], F32, kind="ExternalInput")
    d_indrep = nc.dram_tensor("indrep", [128, HALF], F32, kind="ExternalInput")
    d_labcol = nc.dram_tensor("labcol", [128, HT], F32, kind="ExternalInput")
    d_invc = nc.dram_tensor("invc", [G, 1], F32, kind="ExternalInput")
    d_iotab = nc.dram_tensor("iotab", [128, G], F32, kind="ExternalInput")
    d_iotag = nc.dram_tensor("iotag", [128, 2], F32, kind="ExternalInput")
    d_iota8 = nc.dram_tensor("iota8", [128, T], F32, kind="ExternalInput")
    d_rev7 = nc.dram_tensor("rev7", [128, T], F32, kind="ExternalInput")
    d_wut = nc.dram_tensor("wut", [H, H], F32, kind="ExternalInput")
    d_wpt = nc.dram_tensor("wpt", [H, T], F32, kind="ExternalInput")
    d_bu = nc.dram_tensor("bu", [H, 1], F32, kind="ExternalInput")
    d_bpb = nc.dram_tensor("bpb", [128, T], F32, kind="ExternalInput")
    d_emask = nc.dram_tensor("emask", [128, 1], F32, kind="ExternalInput")
    d_omask = nc.dram_tensor("omask", [128, 1], F32, kind="ExternalInput")
    d_ident = nc.dram_tensor("ident", [128, 128], F32, kind="ExternalInput")

    d_pred = nc.dram_tensor("out_pred", [128, HT], F32, kind="ExternalOutput")
    d_loss = nc.dram_tensor("out_loss", [128, 2], F32, kind="ExternalOutput")

    with tile.TileContext(nc) as tc:
        with (
            tc.tile_pool(name="const", bufs=1) as cp,
            tc.tile_pool(name="eb", bufs=6) as eb,
            tc.tile_pool(name="ob", bufs=4) as ob,
            tc.tile_pool(name="work", bufs=1) as wk,
            tc.tile_pool(name="small", bufs=2) as sm,
            tc.tile_pool(name="pseg", bufs=1, space="PSUM") as pseg,
            tc.tile_pool(name="ptr", bufs=2, space="PSUM") as ptr,
            tc.tile_pool(name="pmm", bufs=2, space="PSUM") as pmm,
        ):
            # ---- constants
            indcol = cp.tile([128, nk], F32, tag="indcol")
            nc.sync.dma_start(indcol[:], d_indcol.ap())
            indrep = cp.tile([128, HALF], F32, tag="indrep")
            nc.sync.dma_start(indrep[:], d_indrep.ap())
            labcol = cp.tile([128, HT], F32, tag="labcol")
            nc.sync.dma_start(labcol[:], d_labcol.ap())
            iotab = cp.tile([128, G], F32, tag="iotab")
            nc.sync.dma_start(iotab[:], d_iotab.ap())
            iotag = cp.tile([128, 2], F32, tag="iotag")
            nc.sync.dma_start(iotag[:], d_iotag.ap())
            iota8 = cp.tile([128, T], F32, tag="iota8")
            nc.sync.dma_start(iota8[:], d_iota8.ap())
            rev7 = cp.tile([128, T], F32, tag="rev7")
            nc.sync.dma_start(rev7[:], d_rev7.ap())
            bpb = cp.tile([128, T], F32, tag="bpb")
            nc.sync.dma_start(bpb[:], d_bpb.ap())
            emask = cp.tile([128, 1], F32, tag="emask")
            nc.sync.dma_start(emask[:], d_emask.ap())
            omask = cp.tile([128, 1], F32, tag="omask")
            nc.sync.dma_start(omask[:], d_omask.ap())
            ident = cp.tile([128, 128], F32, tag="ident")
            nc.sync.dma_start(ident[:], d_ident.ap())
            invc = [cp.tile([128, 1], F32, tag=f"invc{gb}", name=f"invc{gb}") for gb in range(2)]
            for gb in range(2):
                nc.sync.dma_start(invc[gb][:], d_invc.ap()[gb * 128:(gb + 1) * 128, :])
            wut = [cp.tile([128, H], F32R, tag=f"wut{j}", name=f"wut{j}") for j in range(HB)]
            wpt = [cp.tile([128, T], F32R, tag=f"wpt{j}", name=f"wpt{j}") for j in range(HB)]
            bu = [cp.tile([128, 1], F32, tag=f"bu{j}", name=f"bu{j}") for j in range(HB)]
            for j in range(HB):
                sl = slice(j * 128, (j + 1) * 128)
                nc.sync.dma_start(wut[j][:], d_wut.ap()[sl, :].bitcast(F32R))
                nc.sync.dma_start(wpt[j][:], d_wpt.ap()[sl, :].bitcast(F32R))
                nc.sync.dma_start(bu[j][:], d_bu.ap()[sl, :])

            # ---- phase 1: segment sums  (PSUM [g-block, 768] over nk k-tiles)
            psA = [pseg.tile([128, 512], F32, tag=f"psA{gb}", name=f"psA{gb}") for gb in range(2)]
            psB = [pseg.tile([128, 256], F32, tag=f"psB{gb}", name=f"psB{gb}") for gb in range(2)]
            for k in range(nk):
                e_t = eb.tile([128, H], F32R, tag="e")
                nc.sync.dma_start(e_t[:], d_e.ap()[k * 128:(k + 1) * 128, :].bitcast(F32R))
                o_t = ob.tile([128, G], F32R, tag="o")
                nc.vector.tensor_tensor(
                    o_t[:], indcol[:, k:k + 1].broadcast_to([128, G]), iotab[:],
                    op=OP.is_equal)
                st, sp = (k == 0), (k == nk - 1)
                for gb in range(2):
                    osl = o_t[:, gb * 128:(gb + 1) * 128]
                    nc.tensor.matmul(psA[gb][:], osl, e_t[:, 0:512], start=st, stop=sp)
                    nc.tensor.matmul(psB[gb][:], osl, e_t[:, 512:H], start=st, stop=sp)

            # ---- phase 2: means [g, hin] f32  (scale by 1/count)
            means = [wk.tile([128, H], F32, tag=f"means{gb}", name=f"means{gb}") for gb in range(2)]
            for gb in range(2):
                nc.scalar.mul(means[gb][:, 0:512], psA[gb][:], invc[gb][:])
                nc.scalar.mul(means[gb][:, 512:H], psB[gb][:], invc[gb][:])

            # ---- phase 3: transpose -> meansT [hin, g] F32R
            meansT = [wk.tile([128, G], F32R, tag=f"meansT{hb}", name=f"meansT{hb}") for hb in range(HB)]
            for hb in range(HB):
                for gb in range(2):
                    tp = ptr.tile([128, 128], F32, tag="tp")
                    nc.tensor.transpose(tp[:], means[gb][:, hb * 128:(hb + 1) * 128],
                                        ident[:])
                    nc.scalar.copy(meansT[hb][:, gb * 128:(gb + 1) * 128], tp[:])

            # ---- phase 4: L1  h1T[hout-block] = tanh(wut.T @ meansT + bu)
            h1T = [wk.tile([128, G], F32R, tag=f"h1T{j}", name=f"h1T{j}") for j in range(HB)]
            for j in range(HB):
                h1ps = pmm.tile([128, G], F32, tag="mm")
                for hb in range(HB):
                    nc.tensor.matmul(h1ps[:], wut[hb][:, j * 128:(j + 1) * 128],
                                     meansT[hb][:], start=(hb == 0), stop=(hb == HB - 1))
                nc.scalar.activation(h1T[j][:], h1ps[:], AF.Tanh, bias=bu[j][:])

            # ---- phase 5: L2 logits [g-block, 8] + softmax/argmax -> table
            table = [wk.tile([128, 2 + T], F32R, tag=f"table{gb}", name=f"table{gb}") for gb in range(2)]
            for gb in range(2):
                lps = pmm.tile([128, G], F32, tag="mm", name="lps")[:, 0:T]
                for j in range(HB):
                    nc.tensor.matmul(lps[:], h1T[j][:, gb * 128:(gb + 1) * 128],
                                     wpt[j][:], start=(j == 0), stop=(j == HB - 1))
                logits = sm.tile([128, T], F32, tag="logits")
                nc.vector.tensor_tensor(logits[:], lps[:], bpb[:], op=OP.add)
                m = sm.tile([128, 1], F32, tag="m")
                nc.vector.reduce_max(m[:], logits[:], axis=AX.X)
                negm = sm.tile([128, 1], F32, tag="negm")
                nc.scalar.mul(negm[:], m[:], -1.0)
                ex = sm.tile([128, T], F32, tag="ex")
                nc.scalar.activation(ex[:], logits[:], AF.Exp, bias=negm[:])
                ss = sm.tile([128, 1], F32, tag="ss")
                nc.vector.reduce_sum(ss[:], ex[:], axis=AX.X)
                lss = sm.tile([128, 1], F32, tag="lss")
                nc.scalar.activation(lss[:], ss[:], AF.Ln)
                c_t = sm.tile([128, 1], F32, tag="c")
                nc.vector.tensor_add(c_t[:], m[:], lss[:])
                nll = sm.tile([128, T], F32, tag="nll")
                nc.vector.tensor_scalar(nll[:], logits[:], c_t[:], -1.0,
                                        op0=OP.subtract, op1=OP.mult)
                eq = sm.tile([128, T], F32, tag="eq")
                nc.vector.tensor_tensor(eq[:], logits[:],
                                        m[:].broadcast_to([128, T]), op=OP.is_equal)
                sc = sm.tile([128, T], F32, tag="sc")
                nc.vector.tensor_tensor(sc[:], eq[:], rev7[:], op=OP.mult)
                mx = sm.tile([128, 1], F32, tag="mx")
                nc.vector.reduce_max(mx[:], sc[:], axis=AX.X)
                pred = sm.tile([128, 1], F32, tag="pred")
                nc.vector.tensor_scalar(pred[:], mx[:], -1.0, 7.0,
                                        op0=OP.mult, op1=OP.add)
                nc.scalar.copy(table[gb][:, 0:1], pred[:])
                nc.scalar.copy(table[gb][:, 1:1 + T], nll[:])
                nc.scalar.copy(table[gb][:, 1 + T:2 + T], pred[:])

            # ---- phase 7: OT one-hot [g-block partitions, half tokens]
            ot = [wk.tile([128, HALF], F32R, tag=f"ot{gb}", name=f"ot{gb}") for gb in range(2)]
            for gb in range(2):
                nc.vector.tensor_tensor(
                    ot[gb][:], iotag[:, gb:gb + 1].broadcast_to([128, HALF]),
                    indrep[:], op=OP.is_equal)

            # ---- phase 8: token gather + label select
            pred_all = wk.tile([128, HT], F32, tag="pred_all")
            nll_all = wk.tile([128, HT], F32, tag="nll_all")
            for k in range(HT):
                tps = pmm.tile([128, G], F32, tag="mm", name="tps")[:, 0:2 + T]
                ksl = slice(k * 128, (k + 1) * 128)
                nc.tensor.matmul(tps[:], ot[0][:, ksl], table[0][:], start=True, stop=False)
                nc.tensor.matmul(tps[:], ot[1][:, ksl], table[1][:], start=False, stop=True)
                nc.scalar.copy(pred_all[:, k:k + 1], tps[:, 0:1])
                lhot = sm.tile([128, T], F32, tag="lhot")
                nc.vector.tensor_tensor(lhot[:],
                                        labcol[:, k:k + 1].broadcast_to([128, T]),
                                        iota8[:], op=OP.is_equal)
                sel = sm.tile([128, T], F32, tag="sel")
                nc.vector.tensor_tensor(sel[:], tps[:, 1:1 + T], lhot[:], op=OP.mult)
                nc.vector.reduce_sum(nll_all[:, k:k + 1], sel[:], axis=AX.X)
            nc.sync.dma_start(d_pred.ap(), pred_all[:])

            # ---- phase 9: per-partition loss partial sums (host finishes)
            losspair = wk.tile([128, 2], F32, tag="losspair")
            tmp = wk.tile([128, HT], F32, tag="tmpmask")
            nc.vector.tensor_tensor(tmp[:], nll_all[:],
                                    emask[:].broadcast_to([128, HT]), op=OP.mult)
            nc.vector.reduce_sum(losspair[:, 0:1], tmp[:], axis=AX.X)
            tmp2 = wk.tile([128, HT], F32, tag="tmpmask2")
            nc.vector.tensor_tensor(tmp2[:], nll_all[:],
                                    omask[:].broadcast_to([128, HT]), op=OP.mult)
            nc.vector.reduce_sum(losspair[:, 1:2], tmp2[:], axis=AX.X)
            nc.sync.dma_start(d_loss.ap(), losspair[:])

    nc.compile()
    return nc


def _get_nc(nk):
    key = ("nc", nk)
    if key not in _CACHE:
        _CACHE[key] = _build_nc(nk)
    return _CACHE[key]


def kernel(**inputs):
    from concourse.bass_utils import run_bass_kernel_spmd

    E = np.ascontiguousarray(np.asarray(inputs["encoded_states"], dtype=np.float32))
    ind_in = np.asarray(inputs["indicator"])
    lab_in = np.asarray(inputs["ctc_label"])
    W_u = np.asarray(inputs["W_u"], dtype=np.float32)
    b_u = np.asarray(inputs["b_u"], dtype=np.float32)
    W_p = np.asarray(inputs["W_p"], dtype=np.float32)
    b_p = np.asarray(inputs["b_p"], dtype=np.float32)
    ind = ind_in.astype(np.int64)
    lab = lab_in.astype(np.int64)

    nk = S // 128  # 32: full batch per core

    Er = _rnd_fp32r(E.reshape(B * S, H)).reshape(B, S, H)
    wut = _rnd_fp32r(np.ascontiguousarray(W_u.T))
    wpt = _rnd_fp32r(np.ascontiguousarray(W_p.T))

    consts = {
        "iotab": np.broadcast_to(np.arange(G, dtype=np.float32)[None, :], (128, G)).copy(),
        "iotag": (np.arange(128, dtype=np.float32)[:, None]
                  + np.array([0.0, 128.0], np.float32)[None, :]).copy(),
        "iota8": np.broadcast_to(np.arange(T, dtype=np.float32)[None, :], (128, T)).copy(),
        "rev7": np.broadcast_to((7.0 - np.arange(T, dtype=np.float32))[None, :], (128, T)).copy(),
        "wut": wut, "wpt": wpt,
        "bu": b_u.reshape(H, 1).copy(),
        "bpb": np.broadcast_to(b_p[None, :], (128, T)).copy(),
        "emask": (np.arange(128) % 2 == 0).astype(np.float32)[:, None].copy(),
        "omask": (np.arange(128) % 2 == 1).astype(np.float32)[:, None].copy(),
        "ident": np.eye(128, dtype=np.float32),
    }

    in_maps = []
    for c in range(N_CORES):
        b, h = c // 2, c % 2
        half = slice(h * HALF, (h + 1) * HALF)
        cnt = np.bincount(ind[b], minlength=G)
        m = dict(consts)
        m["e"] = Er[b]
        m["indcol"] = np.ascontiguousarray(
            ind[b].astype(np.float32).reshape(nk, 128).T)
        m["indrep"] = np.broadcast_to(
            ind[b, half].astype(np.float32)[None, :], (128, HALF)).copy()
        m["labcol"] = np.ascontiguousarray(
            lab[b, half].astype(np.float32).reshape(HT, 128).T)
        m["invc"] = (1.0 / np.maximum(cnt, 1)).astype(np.float32)[:, None].copy()
        in_maps.append(m)

    nc = _get_nc(nk)
    res = run_bass_kernel_spmd(nc, in_maps, core_ids=list(range(N_CORES)),
                               trace=TRACE)
    _CACHE["last_result"] = res

    pred_f = np.zeros((B, S), np.float32)
    loss = np.zeros(2, np.float64)
    for c in range(N_CORES):
        b, h = c // 2, c % 2
        half = slice(h * HALF, (h + 1) * HALF)
        pred_f[b, half] = res.results[c]["out_pred"].T.reshape(HALF)
        loss += res.results[c]["out_loss"].astype(np.float64).sum(axis=0)

    idt = np.int64 if lab_in.dtype == np.int64 else np.int32
    flatp = pred_f.reshape(-1)
    sep_pred = flatp[0::2].astype(idt)
    tok_pred = flatp[1::2].astype(idt)
    flatl = lab_in.reshape(-1)
    sep_lab = flatl[0::2].copy()
    tok_lab = flatl[1::2].copy()
    sep_loss = np.float32(loss[0] / (B * S / 2))
    tok_loss = np.float32(loss[1] / (B * S / 2))
    return ((sep_loss, sep_pred, sep_lab), (tok_loss, tok_pred, tok_lab))
